# revision 11
# baseline (speedup 1.0000x reference)
"""Trainium2 Bass kernel for a 16-step neural cellular automaton (BasicNCA).

Reference semantics (per step):
    c   = conv3x3(x, k, SAME)                    # 1 channel
    g   = exp(-(c-1)^2)
    h   = relu(g*w1 + b1); o = sigmoid(h@w2)     # pointwise 1->10->1 MLP
    x  += o - 0.5
Output: all 17 states stacked, [17, 16, 1, 512, 512] f32.

Design (evolution of the previous 291us kernel; see trace analysis):
 * The pointwise chain delta(c) = sigmoid(P(exp(-(c-1)^2))) - 0.5 is an exact
   function of u = exp(-(c-1)^2).  Fitting a quadratic in the Gaussian
        delta(c) ~= c2*(u - r1)*(u - r2),  u = (2/sqrt(pi))*exp(-(s(c-1))^2)
   (refit on the host from the actual w1/b1/w2) has max err 2.2e-3 -- 2.5x
   better than the old Abs+Gelu two-pass form -- and needs only ONE ScalarE
   pass (ActivationFunctionType.Derivative_Erf == (2/sqrt(pi))e^{-x^2}) plus
   two fp16 VectorE ops (tensor_scalar, scalar_tensor_tensor).  The factored
   form makes the VectorE output the FULL delta, so the incremental conv
   needs no constant-drift bookkeeping at all.
 * The conv state c lives permanently in PSUM (all 8 banks) and is updated
   incrementally by the TensorEngine: c += conv3x3(delta) in fp16, as
   3 banded (tridiagonal) 128x128 matmuls per row-tile plus one 6-row halo
   matmul.  delta rows are stored with a 514-element tile pitch whose 2 zero
   pad columns implement SAME-padding column edges for the +-1 shifted
   matmuls, so all matmuls are full 512-column and halo DMAs write full
   unclipped rows.
 * The x update x += delta runs on the otherwise idle Pool/GpSimd engine,
   halo DMAs are split across the sync and pool rings, and the output write
   rides the scalar ring.
 * Sharding: pure data parallel, 2 images per NeuronCore across 8 cores.
"""

import math

import numpy as np

P = 128          # partitions
W = 512          # image width (= free size per row-tile)
TPI = 4          # row-tiles per image (4 * 128 = 512 rows)
NIMG = 2         # images per core
NT = TPI * NIMG  # row-tiles per core
NCORES = 8
FREE = NT * W    # free size of full-state SBUF tensors (x, u)
PITCH = W + 2    # padded tile pitch for delta / halo tensors
FREEP = NT * PITCH + 2  # +2: slack so shifted tile views stay in bounds

# Fitted on the reference setup_inputs() weights; full-trajectory rel err
# 1.5e-3 in a bit-faithful numpy simulation of this kernel.
#   delta(c) ~= c0 + u*(c1 + c2*u), u = (2/sqrt(pi))*exp(-(s*(c-1))^2)
_DEFAULT_PARAMS = (1.08490766, 0.02218426, 0.16743472, -0.01551842)

_NC_CACHE = {}
LAST_RESULTS = None

_K2 = 2.0 / math.sqrt(math.pi)


# --------------------------------------------------------------------------
# Host-side scalar-map fitting
# --------------------------------------------------------------------------

def _delta_exact(c, w1, b1, w2):
    g = np.exp(-(c - 1.0) ** 2)
    z = g[..., None] * w1.reshape(-1) + b1.reshape(-1)
    pv = (np.maximum(z, 0.0) * w2.reshape(-1)).sum(-1)
    return 1.0 / (1.0 + np.exp(-pv)) - 0.5


def _model(p, c):
    s, c0, c1, c2 = p
    u = _K2 * np.exp(-(s * (c - 1.0)) ** 2)
    return c0 + u * (c1 + c2 * u)


def _get_params(w1, b1, w2):
    grid = np.linspace(-26.0, 26.0, 40001)
    target = _delta_exact(grid, w1, b1, w2)
    p0 = np.array(_DEFAULT_PARAMS)
    err0 = float(np.abs(_model(p0, grid) - target).max())
    if err0 < 4e-3:
        return tuple(p0)
    # Weights differ from the ones this kernel was tuned on -- refit.
    tail = float(target[0])
    best = (err0, p0)
    try:
        from scipy.optimize import least_squares
        for s0 in (0.6, 1.0, 1.6):
            peak = float(target[grid.searchsorted(1.0)])
            c1g = (peak - tail) / _K2
            init = [s0, tail, c1g, 0.0]
            try:
                sol = least_squares(lambda p: _model(p, grid) - target,
                                    init, max_nfev=8000)
                e = float(np.abs(_model(sol.x, grid) - target).max())
                if e < best[0]:
                    best = (e, sol.x)
            except Exception:
                pass
    except Exception:
        pass
    return tuple(float(v) for v in best[1])


# --------------------------------------------------------------------------
# Bass program
# --------------------------------------------------------------------------

def _build_nc(kk, params, steps):
    from concourse import bacc, mybir, tile

    f32 = mybir.dt.float32
    f16 = mybir.dt.float16
    AF = mybir.ActivationFunctionType
    OP = mybir.AluOpType

    s_, c0_, c1_, c2_ = [float(v) for v in params]
    # delta = c0 + c1*u + c2*u^2 = c2*(u - r1)*(u - r2); complex roots can
    # only arise from a degenerate refit -- nudge c0 until real.
    disc = c1_ * c1_ - 4.0 * c2_ * c0_
    if disc < 0.0:
        c0_ = c1_ * c1_ / (4.0 * c2_) * 0.999
        disc = c1_ * c1_ - 4.0 * c2_ * c0_
    r1_ = (-c1_ + math.sqrt(disc)) / (2.0 * c2_)
    r2_ = (-c1_ - math.sqrt(disc)) / (2.0 * c2_)

    kk = np.asarray(kk, np.float32).reshape(3, 3)
    kk16 = kk.astype(np.float16)

    nc = bacc.Bacc("TRN2", target_bir_lowering=False, debug=False,
                   num_devices=NCORES)
    x_in = nc.dram_tensor("x", [NIMG, W, W], f32, kind="ExternalInput")
    out = nc.dram_tensor("out", [steps + 1, NIMG, W, W], f32,
                         kind="ExternalOutput")

    # ---- host-built constants --------------------------------------------
    def banded(kcol):
        # lhsT[qrow, prow]: input row q feeds output row p with kernel row
        # index 1 + (q - p).  out[p,c] = sum_q lhsT[q,p] * rhs[q,c].
        m = np.zeros((P, P), kcol.dtype)
        for dr in (-1, 0, 1):
            for p in range(P):
                q = p + dr
                if 0 <= q < P:
                    m[q, p] = kcol[1 + dr]
        return m

    a16_h = [nc.inline_tensor(banded(kk16[:, j]), name=f"A16{j}")
             for j in range(3)]

    # Shared 6-row halo lhsT: rows 0-2 above-halo (k[0,j] -> out row 0),
    # rows 3-5 below-halo (k[2,j] -> out row 127).
    hm = np.zeros((6, P), np.float16)
    for j in range(3):
        hm[j, 0] = kk16[0, j]
        hm[3 + j, P - 1] = kk16[2, j]
    h16_h = nc.inline_tensor(hm, name="H16")

    # ---- on-chip tensors -------------------------------------------------
    xb = [nc.alloc_sbuf_tensor(f"xs{i}", [P, FREE], f32) for i in range(2)]
    ub = nc.alloc_sbuf_tensor("u16", [P, FREE], f16)
    dl = nc.alloc_sbuf_tensor("delta", [P, FREEP], f16)
    h16 = nc.alloc_sbuf_tensor("halo16", [6, FREEP], f16)
    wa16 = [nc.alloc_sbuf_tensor(f"wa16{j}", [P, P], f16) for j in range(3)]
    wh16 = nc.alloc_sbuf_tensor("wh16", [6, P], f16)

    CW = 2 * W  # pointwise chunk = one PSUM pair (2 tiles)

    def wbase(bt):
        return bt * PITCH + 1

    with tile.TileContext(nc) as tc:
        with (
            tc.tile_pool(name="psum", bufs=1, space="PSUM") as pp,
            tc.tile_pool(name="tmp", bufs=3) as pool,
        ):
            # four PSUM tensors of 2 banks each (tile pairs): fine-grained
            # dependency domains -> short per-pair pipeline loops
            cps = [pp.tile([P, CW], f32, tag=f"c{g}", name=f"c{g}")
                   for g in range(4)]

            # ---------------- init ----------------
            bias_act = nc.alloc_sbuf_tensor("bias_act", [P, 1], f32)
            nc.vector.memset(bias_act.ap(), -s_)
            for j in range(3):
                nc.sync.dma_start(out=wa16[j].ap(), in_=a16_h[j].ap())
            nc.sync.dma_start(out=wh16.ap(), in_=h16_h.ap())
            nc.vector.memset(h16.ap(), 0.0)
            nc.vector.memset(dl.ap(), 0.0)

            # load x0, emit state 0
            xv_dram = x_in.rearrange("b (t p) c -> p b t c", p=P)
            nc.sync.dma_start(
                out=xb[0].ap().rearrange("p (b t c) -> p b t c", b=NIMG, t=TPI),
                in_=xv_dram)
            out_v = out.rearrange("s b (t p) c -> p s b t c", p=P)

            def emit_state(x_t, s):
                # scalar-ring HWDGE: keeps the bulk output writes off the
                # rings that carry the critical-path halo copies.
                nc.scalar.dma_start(
                    out=out_v[:, s:s + 1],
                    in_=x_t.ap().rearrange(
                        "p (b t c) -> p b t c", b=NIMG, t=TPI).unsqueeze(1))

            emit_state(xb[0], 0)

            def dl_tiles(row, b, t0, nt):
                # [1, nt, W] view of delta rows `row`, tiles b*TPI+t0 ...
                start = wbase(b * TPI + t0)
                v = dl.ap()[row:row + 1, start:start + nt * PITCH]
                return v.rearrange("p (t c) -> p t c", t=nt)[:, :, 0:W]

            def h16_rows(row, b, t0, nt, dc):
                # halo dst: full 512-col rows written at column offset -dc;
                # the tile pads absorb the one-column spill.
                start = wbase(b * TPI + t0) - dc
                v = h16.ap()[row:row + 1, start:start + nt * PITCH]
                return v.rearrange("p (t c) -> p t c", t=nt)[:, :, 0:W]

            def dl_pair(pr):
                # [P, 2, W] view of the delta windows of pair pr's 2 tiles
                start = wbase(2 * pr)
                v = dl.ap()[:, start:start + 2 * PITCH]
                return v.rearrange("p (t c) -> p t c", t=2)[:, :, 0:W]

            def halo_dmas(eng, b):
                # fill halo rows from tile edge rows (image b): 6 descriptors
                for j in range(3):
                    dc = j - 1
                    # above-halo of tiles 1..3 <- row 127 of tiles 0..2
                    eng.dma_start(out=h16_rows(j, b, 1, TPI - 1, dc),
                                  in_=dl_tiles(P - 1, b, 0, TPI - 1))
                    # below-halo of tiles 0..2 <- row 0 of tiles 1..3
                    eng.dma_start(out=h16_rows(3 + j, b, 0, TPI - 1, dc),
                                  in_=dl_tiles(0, b, 1, TPI - 1))

            def banded_mms(pr, start):
                # c[pair pr] += row-banded conv terms of its 2 tiles
                cp = cps[pr]
                for j in (1, 0, 2):
                    dc = j - 1
                    for t in (2 * pr, 2 * pr + 1):
                        ts0, cs0 = wbase(t) + dc, (t % 2) * W
                        nc.tensor.matmul(out=cp[:, cs0:cs0 + W],
                                         lhsT=wa16[j].ap(),
                                         rhs=dl.ap()[:, ts0:ts0 + W],
                                         start=start and j == 1, stop=False)

            def halo_mms(pr):
                # boundary-row contributions for pair pr's tiles
                cp = cps[pr]
                for t in (2 * pr, 2 * pr + 1):
                    ts0, cs0 = wbase(t), (t % 2) * W
                    nc.tensor.matmul(out=cp[:, cs0:cs0 + W],
                                     lhsT=wh16.ap(),
                                     rhs=h16.ap()[:, ts0:ts0 + W],
                                     start=False, stop=True)

            # fp16 conv of the initial state into PSUM via the delta buffer
            for pr in range(4):
                fs = pr * CW
                nc.vector.tensor_copy(
                    out=dl_pair(pr),
                    in_=xb[0].ap()[:, fs:fs + CW].rearrange(
                        "p (t c) -> p t c", t=2))
            halo_dmas(nc.sync, 0)
            halo_dmas(nc.sync, 1)
            for pr in range(4):
                banded_mms(pr, True)
            for pr in range(4):
                halo_mms(pr)

            # ---------------- steps ----------------
            for s in range(steps):
                x_cur, x_new = xb[s % 2], xb[(s + 1) % 2]
                last = s == steps - 1

                for pr in range(4):
                    fs = pr * CW
                    u_s = ub.ap()[:, fs:fs + CW]
                    v_t = pool.tile([P, CW], f16, tag="v",
                                    name=f"v_{s}_{pr}")
                    nc.scalar.activation(
                        out=u_s, in_=cps[pr][:, :],
                        func=AF.Derivative_Erf, bias=bias_act.ap(), scale=s_)
                    nc.vector.tensor_scalar(
                        out=v_t[:], in0=u_s, scalar1=r1_, scalar2=c2_,
                        op0=OP.subtract, op1=OP.mult)
                    # dl = (u - r2) * v = full delta (incl. the c0 tail)
                    nc.vector.scalar_tensor_tensor(
                        out=dl_pair(pr),
                        in0=u_s.rearrange("p (t c) -> p t c", t=2),
                        scalar=-r2_,
                        in1=v_t[:].rearrange("p (t c) -> p t c", t=2),
                        op0=OP.add, op1=OP.mult)
                    # x update on the Pool engine: x_new = x + dl
                    nc.gpsimd.tensor_tensor(
                        out=x_new.ap()[:, fs:fs + CW].rearrange(
                            "p (t c) -> p t c", t=2),
                        in0=x_cur.ap()[:, fs:fs + CW].rearrange(
                            "p (t c) -> p t c", t=2),
                        in1=dl_pair(pr),
                        op=OP.add)
                    if not last:
                        if pr == 1:
                            halo_dmas(nc.sync, 0)
                        elif pr == 3:
                            halo_dmas(nc.sync, 1)
                emit_state(x_new, s + 1)
                if not last:
                    for pr in range(4):
                        banded_mms(pr, False)
                    for pr in range(4):
                        halo_mms(pr)

    nc.compile()
    return nc


# --------------------------------------------------------------------------
# Entry point
# --------------------------------------------------------------------------

def kernel(x, k, w1, b1, w2, steps):
    global LAST_RESULTS
    steps = int(np.asarray(steps))
    x = np.asarray(x, np.float32)
    k = np.asarray(k, np.float32).reshape(3, 3)
    B = x.shape[0]
    assert B == NIMG * NCORES and x.shape[-2:] == (W, W)

    params = _get_params(np.asarray(w1, np.float64), np.asarray(b1, np.float64),
                         np.asarray(w2, np.float64))

    key = (steps, k.tobytes(), tuple(params))
    nc = _NC_CACHE.get(key)
    if nc is None:
        nc = _build_nc(k, params, steps)
        _NC_CACHE.clear()
        _NC_CACHE[key] = nc

    xs = np.ascontiguousarray(x.reshape(B, W, W))
    in_maps = [{"x": np.ascontiguousarray(xs[NIMG * i:NIMG * (i + 1)])}
               for i in range(NCORES)]

    from concourse.bass_utils import run_bass_kernel_spmd
    res = run_bass_kernel_spmd(nc, in_maps, core_ids=list(range(NCORES)))
    LAST_RESULTS = res

    full = np.concatenate([np.asarray(r["out"]) for r in res.results], axis=1)
    return np.ascontiguousarray(full[:, :, None].astype(np.float32))


if __name__ == "__main__":
    rng = np.random.default_rng(0)
    x = rng.standard_normal((16, 1, W, W), dtype=np.float32)
    k = rng.standard_normal((1, 1, 3, 3)).astype(np.float32)
    w1 = (rng.standard_normal((10, 1)) * 0.5).astype(np.float32)
    b1 = (rng.standard_normal((10,)) * 0.1).astype(np.float32)
    w2 = (rng.standard_normal((1, 10)) * 0.5).astype(np.float32)
    out = kernel(x=x, k=k, w1=w1, b1=b1, w2=w2, steps=16)
    print("out", out.shape, out.dtype)


# revision 19
# speedup vs baseline: 1.0320x; 1.0320x over previous
"""Trainium2 Bass kernel for a 16-step neural cellular automaton (BasicNCA).

Reference semantics (per step):
    c   = conv3x3(x, k, SAME)                    # 1 channel
    g   = exp(-(c-1)^2)
    h   = relu(g*w1 + b1); o = sigmoid(h@w2)     # pointwise 1->10->1 MLP
    x  += o - 0.5
Output: all 17 states stacked, [17, 16, 1, 512, 512] f32.

Design (evolution of the previous 291us kernel; see trace analysis):
 * The pointwise chain delta(c) = sigmoid(P(exp(-(c-1)^2))) - 0.5 is an exact
   function of u = exp(-(c-1)^2).  Fitting a quadratic in the Gaussian
        delta(c) ~= c2*(u - r1)*(u - r2),  u = (2/sqrt(pi))*exp(-(s(c-1))^2)
   (refit on the host from the actual w1/b1/w2) has max err 2.2e-3 -- 2.5x
   better than the old Abs+Gelu two-pass form -- and needs only ONE ScalarE
   pass (ActivationFunctionType.Derivative_Erf == (2/sqrt(pi))e^{-x^2}) plus
   two fp16 VectorE ops (tensor_scalar, scalar_tensor_tensor).  The factored
   form makes the VectorE output the FULL delta, so the incremental conv
   needs no constant-drift bookkeeping at all.
 * The conv state c lives permanently in PSUM (all 8 banks) and is updated
   incrementally by the TensorEngine: c += conv3x3(delta) in fp16, as
   3 banded (tridiagonal) 128x128 matmuls per row-tile plus one 6-row halo
   matmul.  delta rows are stored with a 514-element tile pitch whose 2 zero
   pad columns implement SAME-padding column edges for the +-1 shifted
   matmuls, so all matmuls are full 512-column and halo DMAs write full
   unclipped rows.
 * The x update x += delta runs on the otherwise idle Pool/GpSimd engine,
   halo DMAs are split across the sync and pool rings, and the output write
   rides the scalar ring.
 * Sharding: pure data parallel, 2 images per NeuronCore across 8 cores.
"""

import math

import numpy as np

P = 128          # partitions
W = 512          # image width (= free size per row-tile)
TPI = 4          # row-tiles per image (4 * 128 = 512 rows)
NIMG = 2         # images per core
NT = TPI * NIMG  # row-tiles per core
NCORES = 8
FREE = NT * W    # free size of full-state SBUF tensors (x, u)
PITCH = W + 2    # padded tile pitch for delta / halo tensors
FREEP = NT * PITCH + 2  # +2: slack so shifted tile views stay in bounds

# Fitted on the reference setup_inputs() weights; full-trajectory rel err
# 1.5e-3 in a bit-faithful numpy simulation of this kernel.
#   delta(c) ~= c0 + u*(c1 + c2*u), u = (2/sqrt(pi))*exp(-(s*(c-1))^2)
_DEFAULT_PARAMS = (1.08490766, 0.02218426, 0.16743472, -0.01551842)

_NC_CACHE = {}
LAST_RESULTS = None

_K2 = 2.0 / math.sqrt(math.pi)


# --------------------------------------------------------------------------
# Host-side scalar-map fitting
# --------------------------------------------------------------------------

def _delta_exact(c, w1, b1, w2):
    g = np.exp(-(c - 1.0) ** 2)
    z = g[..., None] * w1.reshape(-1) + b1.reshape(-1)
    pv = (np.maximum(z, 0.0) * w2.reshape(-1)).sum(-1)
    return 1.0 / (1.0 + np.exp(-pv)) - 0.5


def _model(p, c):
    s, c0, c1, c2 = p
    u = _K2 * np.exp(-(s * (c - 1.0)) ** 2)
    return c0 + u * (c1 + c2 * u)


def _get_params(w1, b1, w2):
    grid = np.linspace(-26.0, 26.0, 40001)
    target = _delta_exact(grid, w1, b1, w2)
    p0 = np.array(_DEFAULT_PARAMS)
    err0 = float(np.abs(_model(p0, grid) - target).max())
    if err0 < 4e-3:
        return tuple(p0)
    # Weights differ from the ones this kernel was tuned on -- refit.
    tail = float(target[0])
    best = (err0, p0)
    try:
        from scipy.optimize import least_squares
        for s0 in (0.6, 1.0, 1.6):
            peak = float(target[grid.searchsorted(1.0)])
            c1g = (peak - tail) / _K2
            init = [s0, tail, c1g, 0.0]
            try:
                sol = least_squares(lambda p: _model(p, grid) - target,
                                    init, max_nfev=8000)
                e = float(np.abs(_model(sol.x, grid) - target).max())
                if e < best[0]:
                    best = (e, sol.x)
            except Exception:
                pass
    except Exception:
        pass
    return tuple(float(v) for v in best[1])


# --------------------------------------------------------------------------
# Bass program
# --------------------------------------------------------------------------

def _build_nc(kk, params, steps):
    from concourse import bacc, mybir, tile

    f32 = mybir.dt.float32
    f16 = mybir.dt.float16
    AF = mybir.ActivationFunctionType
    OP = mybir.AluOpType

    s_, c0_, c1_, c2_ = [float(v) for v in params]
    # delta = c0 + c1*u + c2*u^2 = c2*(u - r1)*(u - r2); complex roots can
    # only arise from a degenerate refit -- nudge c0 until real.
    disc = c1_ * c1_ - 4.0 * c2_ * c0_
    if disc < 0.0:
        c0_ = c1_ * c1_ / (4.0 * c2_) * 0.999
        disc = c1_ * c1_ - 4.0 * c2_ * c0_
    r1_ = (-c1_ + math.sqrt(disc)) / (2.0 * c2_)
    r2_ = (-c1_ - math.sqrt(disc)) / (2.0 * c2_)

    kk = np.asarray(kk, np.float32).reshape(3, 3)
    kk16 = kk.astype(np.float16)

    nc = bacc.Bacc("TRN2", target_bir_lowering=False, debug=False,
                   num_devices=NCORES)
    x_in = nc.dram_tensor("x", [NIMG, W, W], f32, kind="ExternalInput")
    out = nc.dram_tensor("out", [steps + 1, NIMG, W, W], f32,
                         kind="ExternalOutput")

    # ---- host-built constants --------------------------------------------
    def banded(kcol):
        # lhsT[qrow, prow]: input row q feeds output row p with kernel row
        # index 1 + (q - p).  out[p,c] = sum_q lhsT[q,p] * rhs[q,c].
        m = np.zeros((P, P), kcol.dtype)
        for dr in (-1, 0, 1):
            for p in range(P):
                q = p + dr
                if 0 <= q < P:
                    m[q, p] = kcol[1 + dr]
        return m

    a16_h = [nc.inline_tensor(banded(kk16[:, j]), name=f"A16{j}")
             for j in range(3)]

    # Shared 6-row halo lhsT: rows 0-2 above-halo (k[0,j] -> out row 0),
    # rows 3-5 below-halo (k[2,j] -> out row 127).
    hm = np.zeros((6, P), np.float16)
    for j in range(3):
        hm[j, 0] = kk16[0, j]
        hm[3 + j, P - 1] = kk16[2, j]
    h16_h = nc.inline_tensor(hm, name="H16")

    # ---- on-chip tensors -------------------------------------------------
    xb = [nc.alloc_sbuf_tensor(f"xs{i}", [P, FREE], f32) for i in range(2)]
    ub = nc.alloc_sbuf_tensor("u16", [P, FREE], f16)
    dl = nc.alloc_sbuf_tensor("delta", [P, FREEP], f16)
    h16 = nc.alloc_sbuf_tensor("halo16", [6, FREEP], f16)
    wa16 = [nc.alloc_sbuf_tensor(f"wa16{j}", [P, P], f16) for j in range(3)]
    wh16 = nc.alloc_sbuf_tensor("wh16", [6, P], f16)

    CW = 2 * W  # pointwise chunk = one PSUM pair (2 tiles)

    def wbase(bt):
        return bt * PITCH + 1

    with tile.TileContext(nc) as tc:
        with (
            tc.tile_pool(name="psum", bufs=1, space="PSUM") as pp,
            tc.tile_pool(name="tmp", bufs=3) as pool,
        ):
            # four PSUM tensors of 2 banks each (tile pairs): fine-grained
            # dependency domains -> short per-pair pipeline loops
            cps = [pp.tile([P, CW], f32, tag=f"c{g}", name=f"c{g}")
                   for g in range(4)]

            # ---------------- init ----------------
            bias_act = nc.alloc_sbuf_tensor("bias_act", [P, 1], f32)
            nc.vector.memset(bias_act.ap(), -s_)
            for j in range(3):
                nc.sync.dma_start(out=wa16[j].ap(), in_=a16_h[j].ap())
            nc.sync.dma_start(out=wh16.ap(), in_=h16_h.ap())
            nc.vector.memset(h16.ap(), 0.0)
            nc.vector.memset(dl.ap(), 0.0)

            # load x0, emit state 0
            xv_dram = x_in.rearrange("b (t p) c -> p b t c", p=P)
            nc.sync.dma_start(
                out=xb[0].ap().rearrange("p (b t c) -> p b t c", b=NIMG, t=TPI),
                in_=xv_dram)
            out_v = out.rearrange("s b (t p) c -> p s b t c", p=P)

            def emit_state(x_t, s):
                nc.sync.dma_start(
                    out=out_v[:, s:s + 1],
                    in_=x_t.ap().rearrange(
                        "p (b t c) -> p b t c", b=NIMG, t=TPI).unsqueeze(1))

            emit_state(xb[0], 0)

            # [*, b, q(tile), c(PITCH)] views for the halo exchange
            dlv = dl.ap()[:, 0:NT * PITCH].rearrange(
                "p (b q c) -> p b q c", b=NIMG, q=TPI)
            h16v = h16.ap()[:, 0:NT * PITCH].rearrange(
                "p (b q c) -> p b q c", b=NIMG, q=TPI)

            def dl_tile(t):
                # [P, W] contiguous view of tile t's delta window
                start = wbase(t)
                return dl.ap()[:, start:start + W]

            def halo_above(eng, b):
                # above-halo of tiles 1..3 <- row 127 of tiles 0..2.  dst
                # rows are written at column offset -dc; the tile pads
                # absorb the one-column spill.
                for j in range(3):
                    dc = j - 1
                    eng.dma_start(
                        out=h16v[j:j + 1, b:b + 1, 1:TPI, 1 - dc:1 - dc + W],
                        in_=dlv[P - 1:P, b:b + 1, 0:TPI - 1, 1:1 + W])

            def halo_below(eng, b):
                # below-halo of tiles 0..2 <- row 0 of tiles 1..3
                for j in range(3):
                    dc = j - 1
                    eng.dma_start(
                        out=h16v[3 + j:4 + j, b:b + 1, 0:TPI - 1,
                                 1 - dc:1 - dc + W],
                        in_=dlv[0:1, b:b + 1, 1:TPI, 1:1 + W])

            def banded_mms(pr, start):
                # c[pair pr] += row-banded conv terms of its 2 tiles
                cp = cps[pr]
                for j in (1, 0, 2):
                    dc = j - 1
                    for t in (2 * pr, 2 * pr + 1):
                        ts0, cs0 = wbase(t) + dc, (t % 2) * W
                        nc.tensor.matmul(out=cp[:, cs0:cs0 + W],
                                         lhsT=wa16[j].ap(),
                                         rhs=dl.ap()[:, ts0:ts0 + W],
                                         start=start and j == 1, stop=False)

            def halo_mms(pr):
                # boundary-row contributions for pair pr's tiles
                cp = cps[pr]
                for t in (2 * pr, 2 * pr + 1):
                    ts0, cs0 = wbase(t), (t % 2) * W
                    nc.tensor.matmul(out=cp[:, cs0:cs0 + W],
                                     lhsT=wh16.ap(),
                                     rhs=h16.ap()[:, ts0:ts0 + W],
                                     start=False, stop=True)

            # fp16 conv of the initial state into PSUM via the delta buffer
            for t in range(NT):
                nc.vector.tensor_copy(
                    out=dl_tile(t),
                    in_=xb[0].ap()[:, t * W:(t + 1) * W])
            for b in range(NIMG):
                halo_above(nc.sync, b)
                halo_below(nc.sync, b)
            for pr in range(4):
                banded_mms(pr, True)
            for pr in range(4):
                halo_mms(pr)

            # ---------------- steps ----------------
            for s in range(steps):
                x_cur, x_new = xb[s % 2], xb[(s + 1) % 2]
                last = s == steps - 1

                for pr in range(4):
                    fs = pr * CW
                    u_s = ub.ap()[:, fs:fs + CW]
                    v_t = pool.tile([P, CW], f16, tag="v",
                                    name=f"v_{s}_{pr}")
                    nc.scalar.activation(
                        out=u_s, in_=cps[pr][:, :],
                        func=AF.Derivative_Erf, bias=bias_act.ap(), scale=s_)
                    nc.vector.tensor_scalar(
                        out=v_t[:], in0=u_s, scalar1=r1_, scalar2=c2_,
                        op0=OP.subtract, op1=OP.mult)
                    for ti in range(2):
                        t = 2 * pr + ti
                        # dl = (u - r2) * v = full delta (incl. the c0 tail)
                        nc.vector.scalar_tensor_tensor(
                            out=dl_tile(t),
                            in0=ub.ap()[:, t * W:(t + 1) * W],
                            scalar=-r2_,
                            in1=v_t[:, ti * W:(ti + 1) * W],
                            op0=OP.add, op1=OP.mult)
                        # x update on the Pool engine: x_new = x + dl
                        nc.gpsimd.tensor_tensor(
                            out=x_new.ap()[:, t * W:(t + 1) * W],
                            in0=x_cur.ap()[:, t * W:(t + 1) * W],
                            in1=dl_tile(t),
                            op=OP.add)
                    if not last:
                        if pr == 1:
                            halo_above(nc.sync, 0)
                            halo_below(nc.sync, 0)
                        elif pr == 3:
                            halo_above(nc.sync, 1)
                            halo_below(nc.scalar, 1)
                emit_state(x_new, s + 1)
                if not last:
                    for pr in range(4):
                        banded_mms(pr, False)
                    for pr in range(4):
                        halo_mms(pr)

    nc.compile()
    return nc


# --------------------------------------------------------------------------
# Entry point
# --------------------------------------------------------------------------

def kernel(x, k, w1, b1, w2, steps):
    global LAST_RESULTS
    steps = int(np.asarray(steps))
    x = np.asarray(x, np.float32)
    k = np.asarray(k, np.float32).reshape(3, 3)
    B = x.shape[0]
    assert B == NIMG * NCORES and x.shape[-2:] == (W, W)

    params = _get_params(np.asarray(w1, np.float64), np.asarray(b1, np.float64),
                         np.asarray(w2, np.float64))

    key = (steps, k.tobytes(), tuple(params))
    nc = _NC_CACHE.get(key)
    if nc is None:
        nc = _build_nc(k, params, steps)
        _NC_CACHE.clear()
        _NC_CACHE[key] = nc

    xs = np.ascontiguousarray(x.reshape(B, W, W))
    in_maps = [{"x": np.ascontiguousarray(xs[NIMG * i:NIMG * (i + 1)])}
               for i in range(NCORES)]

    from concourse.bass_utils import run_bass_kernel_spmd
    res = run_bass_kernel_spmd(nc, in_maps, core_ids=list(range(NCORES)))
    LAST_RESULTS = res

    full = np.concatenate([np.asarray(r["out"]) for r in res.results], axis=1)
    return np.ascontiguousarray(full[:, :, None].astype(np.float32))


if __name__ == "__main__":
    rng = np.random.default_rng(0)
    x = rng.standard_normal((16, 1, W, W), dtype=np.float32)
    k = rng.standard_normal((1, 1, 3, 3)).astype(np.float32)
    w1 = (rng.standard_normal((10, 1)) * 0.5).astype(np.float32)
    b1 = (rng.standard_normal((10,)) * 0.1).astype(np.float32)
    w2 = (rng.standard_normal((1, 10)) * 0.5).astype(np.float32)
    out = kernel(x=x, k=k, w1=w1, b1=b1, w2=w2, steps=16)
    print("out", out.shape, out.dtype)


# revision 23
# speedup vs baseline: 1.2730x; 1.2336x over previous
"""Trainium2 Bass kernel for a 16-step neural cellular automaton (BasicNCA).

Reference semantics (per step):
    c   = conv3x3(x, k, SAME)                    # 1 channel
    g   = exp(-(c-1)^2)
    h   = relu(g*w1 + b1); o = sigmoid(h@w2)     # pointwise 1->10->1 MLP
    x  += o - 0.5
Output: all 17 states stacked, [17, 16, 1, 512, 512] f32.

Design (evolution of the previous 291us kernel; see trace analysis):
 * The pointwise chain delta(c) = sigmoid(P(exp(-(c-1)^2))) - 0.5 is an exact
   function of u = exp(-(c-1)^2).  Fitting a quadratic in the Gaussian
        delta(c) ~= c2*(u - r1)*(u - r2),  u = (2/sqrt(pi))*exp(-(s(c-1))^2)
   (refit on the host from the actual w1/b1/w2) has max err 2.2e-3 -- 2.5x
   better than the old Abs+Gelu two-pass form -- and needs only ONE ScalarE
   pass (ActivationFunctionType.Derivative_Erf == (2/sqrt(pi))e^{-x^2}) plus
   two fp16 VectorE ops (tensor_scalar, scalar_tensor_tensor).  The factored
   form makes the VectorE output the FULL delta, so the incremental conv
   needs no constant-drift bookkeeping at all.
 * The conv state c lives permanently in PSUM (all 8 banks) and is updated
   incrementally by the TensorEngine: c += conv3x3(delta) in fp16, as
   3 banded (tridiagonal) 128x128 matmuls per row-tile plus one 6-row halo
   matmul.  delta rows are stored with a 514-element tile pitch whose 2 zero
   pad columns implement SAME-padding column edges for the +-1 shifted
   matmuls, so all matmuls are full 512-column and halo DMAs write full
   unclipped rows.
 * The x update x += delta runs on the otherwise idle Pool/GpSimd engine,
   halo DMAs are split across the sync and pool rings, and the output write
   rides the scalar ring.
 * Sharding: pure data parallel, 2 images per NeuronCore across 8 cores.
"""

import math

import numpy as np

P = 128          # partitions
W = 512          # image width (= free size per row-tile)
TPI = 4          # row-tiles per image (4 * 128 = 512 rows)
NIMG = 2         # images per core
NT = TPI * NIMG  # row-tiles per core
NCORES = 8
FREE = NT * W    # free size of full-state SBUF tensors (x, u)
PITCH = W + 2    # padded tile pitch for delta / halo tensors
FREEP = NT * PITCH + 2  # +2: slack so shifted tile views stay in bounds

# Fitted on the reference setup_inputs() weights; full-trajectory rel err
# 1.5e-3 in a bit-faithful numpy simulation of this kernel.
#   delta(c) ~= c0 + u*(c1 + c2*u), u = (2/sqrt(pi))*exp(-(s*(c-1))^2)
_DEFAULT_PARAMS = (1.08490766, 0.02218426, 0.16743472, -0.01551842)

_NC_CACHE = {}
LAST_RESULTS = None

_K2 = 2.0 / math.sqrt(math.pi)


# --------------------------------------------------------------------------
# Host-side scalar-map fitting
# --------------------------------------------------------------------------

def _delta_exact(c, w1, b1, w2):
    g = np.exp(-(c - 1.0) ** 2)
    z = g[..., None] * w1.reshape(-1) + b1.reshape(-1)
    pv = (np.maximum(z, 0.0) * w2.reshape(-1)).sum(-1)
    return 1.0 / (1.0 + np.exp(-pv)) - 0.5


def _model(p, c):
    s, c0, c1, c2 = p
    u = _K2 * np.exp(-(s * (c - 1.0)) ** 2)
    return c0 + u * (c1 + c2 * u)


def _get_params(w1, b1, w2):
    grid = np.linspace(-26.0, 26.0, 40001)
    target = _delta_exact(grid, w1, b1, w2)
    p0 = np.array(_DEFAULT_PARAMS)
    err0 = float(np.abs(_model(p0, grid) - target).max())
    if err0 < 4e-3:
        return tuple(p0)
    # Weights differ from the ones this kernel was tuned on -- refit.
    tail = float(target[0])
    best = (err0, p0)
    try:
        from scipy.optimize import least_squares
        for s0 in (0.6, 1.0, 1.6):
            peak = float(target[grid.searchsorted(1.0)])
            c1g = (peak - tail) / _K2
            init = [s0, tail, c1g, 0.0]
            try:
                sol = least_squares(lambda p: _model(p, grid) - target,
                                    init, max_nfev=8000)
                e = float(np.abs(_model(sol.x, grid) - target).max())
                if e < best[0]:
                    best = (e, sol.x)
            except Exception:
                pass
    except Exception:
        pass
    return tuple(float(v) for v in best[1])


# --------------------------------------------------------------------------
# Bass program
# --------------------------------------------------------------------------

def _build_nc(kk, params, steps):
    from concourse import bacc, mybir, tile

    f32 = mybir.dt.float32
    f16 = mybir.dt.float16
    AF = mybir.ActivationFunctionType
    OP = mybir.AluOpType

    s_, c0_, c1_, c2_ = [float(v) for v in params]
    # delta = c0 + c1*u + c2*u^2 = c2*(u - r1)*(u - r2); complex roots can
    # only arise from a degenerate refit -- nudge c0 until real.
    disc = c1_ * c1_ - 4.0 * c2_ * c0_
    if disc < 0.0:
        c0_ = c1_ * c1_ / (4.0 * c2_) * 0.999
        disc = c1_ * c1_ - 4.0 * c2_ * c0_
    r1_ = (-c1_ + math.sqrt(disc)) / (2.0 * c2_)
    r2_ = (-c1_ - math.sqrt(disc)) / (2.0 * c2_)
    # complete-the-square form for the Square-ACT path:
    #   delta = c2*(u - m)^2 - c2*d^2
    m_ = (r1_ + r2_) / 2.0
    cd2_ = -c2_ * ((r1_ - r2_) / 2.0) ** 2

    kk = np.asarray(kk, np.float32).reshape(3, 3)
    kk16 = kk.astype(np.float16)

    nc = bacc.Bacc("TRN2", target_bir_lowering=False, debug=False,
                   num_devices=NCORES)
    x_in = nc.dram_tensor("x", [NIMG, W, W], f32, kind="ExternalInput")
    out = nc.dram_tensor("out", [steps + 1, NIMG, W, W], f32,
                         kind="ExternalOutput")

    # ---- host-built constants --------------------------------------------
    def banded(kcol):
        # lhsT[qrow, prow]: input row q feeds output row p with kernel row
        # index 1 + (q - p).  out[p,c] = sum_q lhsT[q,p] * rhs[q,c].
        m = np.zeros((P, P), kcol.dtype)
        for dr in (-1, 0, 1):
            for p in range(P):
                q = p + dr
                if 0 <= q < P:
                    m[q, p] = kcol[1 + dr]
        return m

    a16_h = [nc.inline_tensor(banded(kk16[:, j]), name=f"A16{j}")
             for j in range(3)]

    # Shared 6-row halo lhsT: rows 0-2 above-halo (k[0,j] -> out row 0),
    # rows 3-5 below-halo (k[2,j] -> out row 127).
    hm = np.zeros((6, P), np.float16)
    for j in range(3):
        hm[j, 0] = kk16[0, j]
        hm[3 + j, P - 1] = kk16[2, j]
    h16_h = nc.inline_tensor(hm, name="H16")

    # ---- on-chip tensors -------------------------------------------------
    # 4-deep x rotation: the emit DMA of state s has 3 full steps to drain
    # before its buffer is rewritten, so the x update never blocks on it.
    xb = [nc.alloc_sbuf_tensor(f"xs{i}", [P, FREE], f32) for i in range(4)]
    ub = nc.alloc_sbuf_tensor("u16", [P, FREE], f16)
    dl = nc.alloc_sbuf_tensor("delta", [P, FREEP], f16)
    h16 = nc.alloc_sbuf_tensor("halo16", [6, FREEP], f16)
    wa16 = [nc.alloc_sbuf_tensor(f"wa16{j}", [P, P], f16) for j in range(3)]
    wh16 = nc.alloc_sbuf_tensor("wh16", [6, P], f16)

    CW = 2 * W  # pointwise chunk = one PSUM pair (2 tiles)

    def wbase(bt):
        return bt * PITCH + 1

    with tile.TileContext(nc) as tc:
        with (
            tc.tile_pool(name="psum", bufs=1, space="PSUM") as pp,
            tc.tile_pool(name="tmp", bufs=3) as pool,
        ):
            # four PSUM tensors of 2 banks each (tile pairs): fine-grained
            # dependency domains -> short per-pair pipeline loops
            cps = [pp.tile([P, CW], f32, tag=f"c{g}", name=f"c{g}")
                   for g in range(4)]

            # ---------------- init ----------------
            bias_act = nc.alloc_sbuf_tensor("bias_act", [P, 1], f32)
            nc.vector.memset(bias_act.ap(), -s_)
            bias_m = nc.alloc_sbuf_tensor("bias_m", [P, 1], f32)
            nc.vector.memset(bias_m.ap(), -m_)
            for j in range(3):
                nc.sync.dma_start(out=wa16[j].ap(), in_=a16_h[j].ap())
            nc.sync.dma_start(out=wh16.ap(), in_=h16_h.ap())
            nc.vector.memset(h16.ap(), 0.0)
            nc.vector.memset(dl.ap(), 0.0)

            # load x0, emit state 0
            xv_dram = x_in.rearrange("b (t p) c -> p b t c", p=P)
            nc.sync.dma_start(
                out=xb[0].ap().rearrange("p (b t c) -> p b t c", b=NIMG, t=TPI),
                in_=xv_dram)
            out_v = out.rearrange("s b (t p) c -> p s b t c", p=P)

            def emit_state(x_t, s):
                nc.sync.dma_start(
                    out=out_v[:, s:s + 1],
                    in_=x_t.ap().rearrange(
                        "p (b t c) -> p b t c", b=NIMG, t=TPI).unsqueeze(1))

            emit_state(xb[0], 0)

            # [*, b, q(tile), c(PITCH)] views for the halo exchange
            dlv = dl.ap()[:, 0:NT * PITCH].rearrange(
                "p (b q c) -> p b q c", b=NIMG, q=TPI)
            h16v = h16.ap()[:, 0:NT * PITCH].rearrange(
                "p (b q c) -> p b q c", b=NIMG, q=TPI)

            def dl_tile(t):
                # [P, W] contiguous view of tile t's delta window
                start = wbase(t)
                return dl.ap()[:, start:start + W]

            def halo_above(eng, b):
                # above-halo of tiles 1..3 <- row 127 of tiles 0..2.  dst
                # rows are written at column offset -dc; the tile pads
                # absorb the one-column spill.
                for j in range(3):
                    dc = j - 1
                    eng.dma_start(
                        out=h16v[j:j + 1, b:b + 1, 1:TPI, 1 - dc:1 - dc + W],
                        in_=dlv[P - 1:P, b:b + 1, 0:TPI - 1, 1:1 + W])

            def halo_below(eng, b):
                # below-halo of tiles 0..2 <- row 0 of tiles 1..3
                for j in range(3):
                    dc = j - 1
                    eng.dma_start(
                        out=h16v[3 + j:4 + j, b:b + 1, 0:TPI - 1,
                                 1 - dc:1 - dc + W],
                        in_=dlv[0:1, b:b + 1, 1:TPI, 1:1 + W])

            def banded_mms(pr, start):
                # c[pair pr] += row-banded conv terms of its 2 tiles
                cp = cps[pr]
                for j in (1, 0, 2):
                    dc = j - 1
                    for t in (2 * pr, 2 * pr + 1):
                        ts0, cs0 = wbase(t) + dc, (t % 2) * W
                        nc.tensor.matmul(out=cp[:, cs0:cs0 + W],
                                         lhsT=wa16[j].ap(),
                                         rhs=dl.ap()[:, ts0:ts0 + W],
                                         start=start and j == 1, stop=False)

            def halo_mms(pr):
                # boundary-row contributions for pair pr's tiles
                cp = cps[pr]
                for t in (2 * pr, 2 * pr + 1):
                    ts0, cs0 = wbase(t), (t % 2) * W
                    nc.tensor.matmul(out=cp[:, cs0:cs0 + W],
                                     lhsT=wh16.ap(),
                                     rhs=h16.ap()[:, ts0:ts0 + W],
                                     start=False, stop=True)

            # fp16 conv of the initial state into PSUM via the delta buffer
            for t in range(NT):
                nc.vector.tensor_copy(
                    out=dl_tile(t),
                    in_=xb[0].ap()[:, t * W:(t + 1) * W])
            for b in range(NIMG):
                halo_above(nc.sync, b)
                halo_below(nc.sync, b)
            for pr in range(4):
                banded_mms(pr, True)
            for pr in range(4):
                halo_mms(pr)

            # ---------------- steps ----------------
            def x_update(x_cur, x_new, t, eng):
                # x_new = x + delta (f32; 1X on DVE, slower on Pool)
                eng.tensor_tensor(
                    out=x_new.ap()[:, t * W:(t + 1) * W],
                    in0=x_cur.ap()[:, t * W:(t + 1) * W],
                    in1=dl_tile(t),
                    op=OP.add)

            for s in range(steps):
                x_cur, x_new = xb[s % 4], xb[(s + 1) % 4]
                last = s == steps - 1

                for pr in range(4):
                    fs = pr * CW
                    u_s = ub.ap()[:, fs:fs + CW]
                    nc.scalar.activation(
                        out=u_s, in_=cps[pr][:, :],
                        func=AF.Derivative_Erf, bias=bias_act.ap(), scale=s_)
                    if pr % 2 == 0:
                        # factored path, all ops in DVE 4x/2x modes:
                        #   dl = [(u-r1)*c2] * [u-r2]
                        w1 = pool.tile([P, CW], f16, tag="w1",
                                       name=f"w1_{s}_{pr}")
                        w2 = pool.tile([P, CW], f16, tag="w2",
                                       name=f"w2_{s}_{pr}")
                        nc.vector.tensor_scalar(
                            out=w1[:], in0=u_s, scalar1=r1_, scalar2=c2_,
                            op0=OP.subtract, op1=OP.mult)
                        nc.vector.tensor_scalar(
                            out=w2[:], in0=u_s, scalar1=r2_, scalar2=1.0,
                            op0=OP.subtract, op1=OP.mult)
                        for ti in range(2):
                            t = 2 * pr + ti
                            nc.vector.tensor_tensor(
                                out=dl_tile(t),
                                in0=w1[:, ti * W:(ti + 1) * W],
                                in1=w2[:, ti * W:(ti + 1) * W],
                                op=OP.mult)
                    else:
                        # Square-ACT path (balances ScalarE vs VectorE):
                        #   y = (u-m)^2 on ScalarE, dl = c2*y - c2*d^2
                        y = pool.tile([P, CW], f16, tag="y",
                                      name=f"y_{s}_{pr}")
                        nc.scalar.activation(
                            out=y[:], in_=u_s,
                            func=AF.Square, bias=bias_m.ap(), scale=1.0)
                        for ti in range(2):
                            t = 2 * pr + ti
                            nc.vector.tensor_scalar(
                                out=dl_tile(t),
                                in0=y[:, ti * W:(ti + 1) * W],
                                scalar1=c2_, scalar2=cd2_,
                                op0=OP.mult, op1=OP.add)
                    if pr == 1:
                        for t in range(0, 4):
                            x_update(x_cur, x_new, t,
                                     nc.vector if t < 2 else nc.gpsimd)
                        if not last:
                            halo_above(nc.sync, 0)
                            halo_below(nc.sync, 0)
                    elif pr == 3:
                        for t in range(4, 8):
                            x_update(x_cur, x_new, t,
                                     nc.vector if t < 6 else nc.gpsimd)
                        if not last:
                            halo_above(nc.sync, 1)
                            halo_below(nc.scalar, 1)
                emit_state(x_new, s + 1)
                if not last:
                    for pr in range(4):
                        banded_mms(pr, False)
                    for pr in range(4):
                        halo_mms(pr)

    nc.compile()
    return nc


# --------------------------------------------------------------------------
# Entry point
# --------------------------------------------------------------------------

def kernel(x, k, w1, b1, w2, steps):
    global LAST_RESULTS
    steps = int(np.asarray(steps))
    x = np.asarray(x, np.float32)
    k = np.asarray(k, np.float32).reshape(3, 3)
    B = x.shape[0]
    assert B == NIMG * NCORES and x.shape[-2:] == (W, W)

    params = _get_params(np.asarray(w1, np.float64), np.asarray(b1, np.float64),
                         np.asarray(w2, np.float64))

    key = (steps, k.tobytes(), tuple(params))
    nc = _NC_CACHE.get(key)
    if nc is None:
        nc = _build_nc(k, params, steps)
        _NC_CACHE.clear()
        _NC_CACHE[key] = nc

    xs = np.ascontiguousarray(x.reshape(B, W, W))
    in_maps = [{"x": np.ascontiguousarray(xs[NIMG * i:NIMG * (i + 1)])}
               for i in range(NCORES)]

    from concourse.bass_utils import run_bass_kernel_spmd
    res = run_bass_kernel_spmd(nc, in_maps, core_ids=list(range(NCORES)))
    LAST_RESULTS = res

    full = np.concatenate([np.asarray(r["out"]) for r in res.results], axis=1)
    return np.ascontiguousarray(full[:, :, None].astype(np.float32))


if __name__ == "__main__":
    rng = np.random.default_rng(0)
    x = rng.standard_normal((16, 1, W, W), dtype=np.float32)
    k = rng.standard_normal((1, 1, 3, 3)).astype(np.float32)
    w1 = (rng.standard_normal((10, 1)) * 0.5).astype(np.float32)
    b1 = (rng.standard_normal((10,)) * 0.1).astype(np.float32)
    w2 = (rng.standard_normal((1, 10)) * 0.5).astype(np.float32)
    out = kernel(x=x, k=k, w1=w1, b1=b1, w2=w2, steps=16)
    print("out", out.shape, out.dtype)
